# revision 19
# baseline (speedup 1.0000x reference)
"""Distributed Trainium2 Bass kernel for gnn_message_passing (8 NeuronCores).

Strategy (atom/target sharding, graph-parallel), v2:
  - Atoms are partitioned into 8 contiguous target shards (3750 atoms each).
  - Each edge type's edges are sorted by target and packed into variable-width
    target windows chosen greedily so that every window has <=128 edges on
    EVERY core and <=W_MAX targets; one 128-edge chunk per window.
  - Per window: gather source rows (dma_gather, grouped), expand the edge
    outer-product Z = F (x) xn in ONE broadcast tensor-op (DVE/Pool
    alternating), segment-sum via a one-hot scatter matmul into PSUM
    (G^T orientation), drain to SBUF fp16, then contract with the packed
    edge-network weight (out = G @ Wr) on the TensorEngine.
  - The one-hot scatter matrices S are built ON DEVICE (iota==tloc compare),
    so only compact per-edge data (idx/features/target-offsets) is shipped.
  - atom_features are shipped SHARDED and allgathered on device; the GRU
    reads its inputs via two bulk DMAs per step instead of per-window loads.
  - After bond/angle message sub-steps and each GRU step, cores exchange
    shard rows with an AllGather so the next sub-step's random-source
    gathers see the full table.
"""

import os
import sys

sys.path.insert(0, "/opt/trn_rl_repo")

import numpy as np

import concourse.bass as bass
import concourse.mybir as mybir
import concourse.tile as tile
from concourse import library_config
from concourse.library_overlay import lower_extended_insts
from concourse.tile_rust import add_dep_helper
from concourse.masks import make_identity
from concourse.bass_utils import run_bass_kernel_spmd

# ---------------------------------------------------------------- tile patch
# This walrus build accepts at most ONE sync wait per instruction; spread
# extra waits across same-engine nops placed right before the instruction.
from concourse.tile import TileContext
from concourse.vector_clock import ScopedClock

_orig_drain_and_barrier = TileContext._drain_and_barrier


def _patched_drain_and_barrier(self, tick_clock, wait_clock):
    nc = self.nc
    probe = nc.sync.nop(nofuse=True)
    wait_clock.add_sem_waits(probe.ins, ScopedClock({None: tick_clock.global_clock}))
    si = probe.ins.sync_info
    waits = list(si.on_wait) if si is not None and si.on_wait else []
    if si is not None:
        si.on_wait = waits[:1]
    for w in waits[1:]:
        nop = nc.sync.nop(nofuse=True)
        nop.ins.sync_info = mybir.SyncInfo(on_wait=[w], on_update=[])
    nc.sync.drain()
    nc.all_engine_barrier()
    popped = nc._tile_sem_poison_stack.pop()
    assert popped is self._sem_poison
    nc.clear_and_free_semaphores(list(self.sems.allocated().values()))
    nc.all_engine_barrier()


TileContext._drain_and_barrier = _patched_drain_and_barrier


def _split_multi_waits(nc, max_waits=1):
    n = 0
    for f in nc.m.functions:
        for bb in f.blocks:
            out = []
            for inst in bb.instructions:
                si = inst.sync_info
                if si is not None and si.on_wait and len(si.on_wait) > max_waits:
                    waits = list(si.on_wait)
                    for w in waits[:-max_waits]:
                        nop = mybir.InstNoOp(
                            name=f"wsplit-{nc.next_id()}", ins=[], outs=[],
                            engine=inst.engine)
                        nop.sync_info = mybir.SyncInfo(on_wait=[w], on_update=[])
                        try:
                            nc.register_instruction(nop, overwrite=True)
                        except Exception:
                            pass
                        out.append(nop)
                        n += 1
                    si.on_wait = waits[-max_waits:]
                out.append(inst)
            bb.instructions = out
    return n


# ------------------------------------------------------------------- config
NCORES = 8
STEPS = int(os.environ.get("KSTEPS", "4"))
D = 64
BF16 = mybir.dt.float16  # fp16: 10-bit mantissa, full PE rate
F32 = mybir.dt.float32
I16 = mybir.dt.int16

# max targets per window; gt PSUM tile is [128, 1536] (3 banks): block b at
# cols b*128, stage-B accumulator at cols 1472:1536
W_MAX = {"bond": 128, "angle": 128, "dihedral": 128}
GATHER_GROUP = 6  # windows (128-edge chunks) per dma_gather call
OUT_COL = 1472  # stage-B accumulator column inside the gt PSUM tile
POOL_Z_MOD = 10  # zi % POOL_Z_MOD < POOL_Z_CNT -> per-k z on Pool
POOL_Z_CNT = int(os.environ.get("KPOOLZ", "0"))

_last_results = {}  # test.py introspection


# --------------------------------------------------------------- host prep
def _wrap_idx16(idx):
    """dma_gather index layout, un-replicated: [16, n/16]; partition p holds
    idx[p::16]. Replicated to 128 partitions on device."""
    n = len(idx)
    assert n % 128 == 0
    w = np.zeros((16, n // 16), dtype=np.int16)
    for p in range(16):
        w[p, :] = idx[p::16]
    return w


def _prep_type(pair_idx, feat, n_atoms, shard, wmax):
    """Greedy shared window schedule + per-core packed tensors for one type.

    Windows are contiguous target ranges with <=128 edges on every core and
    width <= wmax; exactly one 128-edge chunk per window per core.
    Returns (sched [(row0,width)], per_core list of dicts, nch, util).
    """
    E, f = feat.shape
    fb = f + 1
    tgt = pair_idx[:, 0].astype(np.int64)
    src = pair_idx[:, 1].astype(np.int64)
    core = tgt // shard
    tloc = tgt % shard

    # per-core degree per local target: [NCORES, shard]
    deg = np.zeros((NCORES, shard), dtype=np.int64)
    np.add.at(deg, (core, tloc), 1)

    sched = []
    row0 = 0
    cnt = np.zeros(NCORES, dtype=np.int64)
    start = 0
    for t in range(shard):
        d = deg[:, t]
        if t > start and ((cnt + d) > 128).any() or (t - start) >= wmax:
            sched.append((start, t - start))
            start = t
            cnt = d.copy()
        else:
            cnt += d
    sched.append((start, shard - start))
    nch = len(sched)
    row0s = np.array([r for (r, _) in sched], dtype=np.int64)

    per_core = []
    for c in range(NCORES):
        sel = core == c
        t_c = tloc[sel]
        s_c = src[sel]
        f_c = feat[sel]
        w_c = np.searchsorted(row0s, t_c, side="right") - 1
        idx_all = np.zeros(nch * 128, dtype=np.int16)
        F_all = np.zeros((128, nch, fb), dtype=np.float16)
        T_all = np.full((128, nch), 999.0, dtype=np.float32)
        eorder = np.argsort(w_c, kind="stable")
        t_c, s_c, f_c, w_c = t_c[eorder], s_c[eorder], f_c[eorder], w_c[eorder]
        b = np.concatenate([[0], np.cumsum(np.bincount(w_c, minlength=nch))])
        for w in range(nch):
            lo, hi = b[w], b[w + 1]
            n_e = hi - lo
            assert n_e <= 128, (w, n_e)
            ps = np.arange(n_e)
            idx_all[w * 128 + ps] = s_c[lo:hi].astype(np.int16)
            F_all[ps, w, :f] = f_c[lo:hi]
            F_all[ps, w, f] = 1.0
            T_all[ps, w] = t_c[lo:hi] - row0s[w]
        per_core.append(dict(idx=_wrap_idx16(idx_all), F=F_all, T=T_all))
    util = E / (nch * 128 * NCORES)
    return sched, per_core, nch, util


def _pack_wr(Wt, bt, f):
    """Wr[k*64+j, i] = Wt[k, i*64+j]; bias block at k=f; pad to cb*128 rows."""
    cb = ((f + 1) * D + 127) // 128
    Wr = np.zeros((cb * 128, D), dtype=np.float32)
    Kr = Wt.reshape(f, D, D)
    for k in range(f):
        Wr[k * D:(k + 1) * D, :] = Kr[k].T
    Wr[f * D:(f + 1) * D, :] = bt.reshape(D, D).T
    # SBUF layout [128, cb, 64]
    return np.ascontiguousarray(
        Wr.reshape(cb, 128, D).transpose(1, 0, 2)).astype(np.float16)


# ------------------------------------------------------------ kernel build
def _build(n_atoms, scheds, nchs, fdims, trace_label=""):
    shard = n_atoms // NCORES
    ntile = (shard + 127) // 128  # GRU column-tiles (bulk [128, ntile, 64])
    nfull = shard // 128
    tail = shard - nfull * 128
    nc = bass.Bass(num_devices=NCORES)
    types = ["bond", "angle", "dihedral"]
    cbs = {t: ((fdims[t] + 1) * D + 127) // 128 for t in types}
    for t in types:
        assert cbs[t] * 128 <= OUT_COL, t

    # ---- I/O tensors
    x0_shard = nc.dram_tensor("x0_shard", [shard, D], F32, kind="ExternalInput")
    iota_in = nc.dram_tensor("iota_in", [128, 128], BF16, kind="ExternalInput")
    ins = {}
    for t in types:
        f = fdims[t]
        ins[t] = dict(
            idx=nc.dram_tensor(f"{t}_idx", [16, nchs[t] * 8], I16, kind="ExternalInput"),
            F=nc.dram_tensor(f"{t}_F", [128, nchs[t], f + 1], BF16, kind="ExternalInput"),
            T=nc.dram_tensor(f"{t}_T", [128, nchs[t]], F32, kind="ExternalInput"),
            Wr=nc.dram_tensor(f"{t}_Wr", [128, cbs[t], D], BF16, kind="ExternalInput"),
        )
    wiT = nc.dram_tensor("wiT", [65, 3 * D], BF16, kind="ExternalInput")
    whT = nc.dram_tensor("whT", [65, 3 * D], BF16, kind="ExternalInput")
    out_shard = nc.dram_tensor("out_shard", [shard, D], BF16, kind="ExternalOutput")

    # ---- internal DRAM: allgathered tables per step
    tables = {}
    tables["x0"] = nc.dram_tensor("tab_x0", [n_atoms, D], F32, addr_space="Shared")
    for s in range(STEPS):
        for t in ("a1", "a2"):
            tables[(s, t)] = nc.dram_tensor(
                f"tab_{t}_{s}", [n_atoms, D], F32, addr_space="Shared")
        if s < STEPS - 1:
            tables[(s, "h")] = nc.dram_tensor(
                f"tab_h_{s}", [n_atoms, D], F32, addr_space="Shared")

    with tile.TileContext(nc) as tc:
        with (
            tc.tile_pool(name="const", bufs=1) as cpool,
            tc.tile_pool(name="work", bufs=3) as pool,
            tc.tile_pool(name="dram", bufs=1, space="DRAM") as dpool,
            tc.tile_pool(name="psum", bufs=2, space="PSUM") as psum,
        ):
            nc.gpsimd.load_library(library_config.mlp)

            _reg_cache = {}

            def idx_reg(v):
                if v not in _reg_cache:
                    _reg_cache[v] = nc.gpsimd.to_reg(v)
                return _reg_cache[v]

            # ---- persistent SBUF constants
            ct = {}
            iota_t = cpool.tile([128, 128], BF16, name="iota_t")
            nc.sync.dma_start(out=iota_t[:], in_=iota_in[:])
            for t in types:
                f = fdims[t]
                fb = f + 1
                it = ins[t]
                nch = nchs[t]
                idx_t = cpool.tile([128, nch * 8], I16, name=f"idx_{t}")
                nc.sync.dma_start(out=idx_t[0:16, :], in_=it["idx"][:])
                for k in range(1, 8):
                    nc.sync.dma_start(out=idx_t[16 * k:16 * (k + 1), :],
                                      in_=idx_t[0:16, :])
                F_t = cpool.tile([128, nch, fb], BF16, name=f"F_{t}")
                nc.sync.dma_start(out=F_t[:], in_=it["F"][:])
                T_t = cpool.tile([128, nch], F32, name=f"T_{t}")
                nc.sync.dma_start(out=T_t[:], in_=it["T"][:])
                Wr_t = cpool.tile([128, cbs[t], D], BF16, name=f"Wr_{t}")
                nc.sync.dma_start(out=Wr_t[:], in_=it["Wr"][:])
                F32_t = cpool.tile([128, nch, fb], F32, name=f"F32_{t}")
                nc.vector.tensor_copy(F32_t[:], F_t[:])
                # build one-hot scatter matrices on device: S[:, ci, :] =
                # (iota == tloc[:, ci]) as fp16
                S_t = cpool.tile([128, nch, 128], BF16, name=f"S_{t}")
                for ci in range(nch):
                    nc.vector.tensor_scalar(
                        out=S_t[:, ci, :], in0=iota_t[:],
                        scalar1=T_t[:, ci:ci + 1], scalar2=None,
                        op0=mybir.AluOpType.is_equal)
                ct[t] = dict(idx=idx_t, F=F_t, F32=F32_t, S=S_t, Wr=Wr_t)
            wiT_t = cpool.tile([65, 3 * D], BF16, name="wiT_t")
            nc.sync.dma_start(out=wiT_t[:], in_=wiT[:])
            whT_t = cpool.tile([65, 3 * D], BF16, name="whT_t")
            nc.sync.dma_start(out=whT_t[:], in_=whT[:])
            ident = cpool.tile([128, 128], F32, name="ident")
            make_identity(nc, ident[:])

            # shard-sized DRAM buffers (dep-tracked)
            a_shard = {}
            h_shard = {}
            for s in range(STEPS):
                for t in ("a1", "a2", "a3"):
                    a_shard[(s, t)] = dpool.tile([shard, D], F32, name=f"a_{t}_{s}")
                if s < STEPS - 1:
                    h_shard[s] = dpool.tile([shard, D], F32, name=f"h_{s}")
            x0s = dpool.tile([shard, D], F32, name="x0s")
            nc.sync.dma_start(out=x0s[:], in_=x0_shard[:])

            def allgather(shard_tile, full_tensor):
                return nc.gpsimd.collective_compute(
                    "AllGather",
                    mybir.AluOpType.bypass,
                    replica_groups=[list(range(NCORES))],
                    ins=[shard_tile[:]],
                    outs=[full_tensor[:]],
                )

            cc0 = allgather(x0s, tables["x0"])

            def message_substep(step, t, src_table_ap, dst_shard, gather_deps):
                f = fdims[t]
                fb = f + 1
                cb = cbs[t]
                sched = scheds[t]
                nch = nchs[t]
                c = ct[t]
                my_gathers = []
                groups = [sched[i:i + GATHER_GROUP]
                          for i in range(0, nch, GATHER_GROUP)]
                ch_base = 0
                zi = 0
                for g in groups:
                    g_ch = len(g)
                    xn = pool.tile([128, GATHER_GROUP, D], F32, tag="xn",
                                   name=f"xn_{t}_{step}_{g[0][0]}", bufs=3)
                    gather = nc.gpsimd.dma_gather(
                        out_ap=xn[:, 0:g_ch, :],
                        in_ap=src_table_ap,
                        idxs_ap=c["idx"][:, ch_base * 8:(ch_base + g_ch) * 8],
                        num_idxs=g_ch * 128,
                        num_idxs_reg=idx_reg(g_ch * 128),
                        elem_size=D,
                    )
                    for dep in gather_deps:
                        add_dep_helper(gather.ins, dep.ins, reason="gather after AG")
                    my_gathers.append(gather)
                    for gci, (row0, width) in enumerate(g):
                        ci = ch_base + gci
                        W = width
                        # z = F (x) xn: one broadcast op on DVE, or per-k
                        # tensor_scalar ops on Pool (TensorTensor is not in
                        # the Pool mlp library)
                        z = pool.tile([128, fb, D], BF16, tag=f"z_{t}",
                                      name=f"z_{t}_{step}_{ci}", bufs=3)
                        if zi % POOL_Z_MOD < POOL_Z_CNT:
                            for k in range(fb):
                                nc.gpsimd.tensor_scalar_mul(
                                    z[:, k, :], xn[:, gci, :],
                                    c["F32"][:, ci, k:k + 1])
                        else:
                            xn_b = xn[:, gci, :].unsqueeze(1).broadcast_to([128, fb, D])
                            F_b = c["F"][:, ci, :].unsqueeze(2).broadcast_to([128, fb, D])
                            nc.vector.tensor_mul(z[:], xn_b, F_b)
                        zi += 1
                        zf = z[:].rearrange("p k j -> p (k j)")
                        gt = psum.tile([128, 1536], F32, tag="gt",
                                       name=f"gt_{t}_{step}_{ci}")
                        for b in range(cb):
                            cw = min(128, fb * D - b * 128)
                            nc.tensor.matmul(
                                gt[:cw, b * 128:b * 128 + W],
                                lhsT=zf[:, b * 128:b * 128 + cw],
                                rhs=c["S"][:, ci, 0:W],
                                start=True, stop=True)
                        # drain G^T to SBUF fp16: one op per bank (4 blocks),
                        # bias block (cw=64) separately
                        gtsb = pool.tile([128, cb, 128], BF16, tag=f"gtsb_{t}",
                                         name=f"gtsb_{t}_{step}_{ci}", bufs=2)
                        nfb = fb * D // 128  # full blocks
                        for b0 in range(0, nfb, 4):
                            nb = min(4, nfb - b0)
                            nc.scalar.activation(
                                gtsb[:, b0:b0 + nb, 0:W],
                                gt[:, b0 * 128:(b0 + nb) * 128].rearrange(
                                    "p (c w) -> p c w", w=128)[:, :, 0:W],
                                mybir.ActivationFunctionType.Copy)
                        if fb * D % 128:  # bias block, 64 rows
                            nc.scalar.activation(
                                gtsb[0:64, cb - 1, 0:W],
                                gt[0:64, (cb - 1) * 128:(cb - 1) * 128 + W],
                                mybir.ActivationFunctionType.Copy)
                        # out-mm: out[tl, i] = sum_b G^T_b[:, tl].T @ Wr_b
                        # accumulates into the gt tile's tail bank
                        pmm = None
                        for b in range(cb):
                            cw = min(128, fb * D - b * 128)
                            mm = nc.tensor.matmul(
                                gt[:W, OUT_COL:OUT_COL + D],
                                lhsT=gtsb[:cw, b, 0:W],
                                rhs=c["Wr"][:cw, b, :],
                                start=(b == 0), stop=(b == cb - 1))
                            if pmm is not None:
                                add_dep_helper(mm.ins, pmm.ins, reason="psum accum")
                            pmm = mm
                        osb = pool.tile([128, D], F32, tag="osb",
                                        name=f"osb_{t}_{step}_{ci}", bufs=3)
                        nc.scalar.activation(osb[:W, :], gt[:W, OUT_COL:OUT_COL + D],
                                             mybir.ActivationFunctionType.Copy)
                        nc.sync.dma_start(
                            out=dst_shard[row0:row0 + W, :],
                            in_=osb[:W, :])
                    ch_base += g_ch
                return my_gathers

            def bulk_load(dst_tile, src_ap):
                """DRAM [shard, D] -> SBUF [128, ntile, D] (atom a ->
                partition a%128, col a//128)."""
                if tail:
                    nc.vector.memset(dst_tile[:, ntile - 1, :], 0.0)
                nc.sync.dma_start(
                    out=dst_tile[:, 0:nfull, :],
                    in_=src_ap[0:nfull * 128, :].rearrange(
                        "(c p) j -> p c j", p=128))
                if tail:
                    nc.sync.dma_start(
                        out=dst_tile[0:tail, ntile - 1, :],
                        in_=src_ap[nfull * 128:shard, :])

            def bulk_store(src_tile, dst_ap):
                nc.sync.dma_start(
                    out=dst_ap[0:nfull * 128, :].rearrange("(c p) j -> p c j", p=128),
                    in_=src_tile[:, 0:nfull, :])
                if tail:
                    nc.sync.dma_start(
                        out=dst_ap[nfull * 128:shard, :],
                        in_=src_tile[0:tail, ntile - 1, :])

            def gru_step(step, a3, hsrc_ap, dst_h, dst_out):
                x_all = pool.tile([128, ntile, D], F32, tag="gxa",
                                  name=f"gxa_{step}", bufs=1)
                bulk_load(x_all, a3)
                h_all = pool.tile([128, ntile, D], F32, tag="gha",
                                  name=f"gha_{step}", bufs=1)
                bulk_load(h_all, hsrc_ap)
                hp_all = pool.tile([128, ntile, D],
                                   BF16 if dst_out is not None else F32,
                                   tag=f"ghp{int(dst_out is not None)}",
                                   name=f"ghp_{step}", bufs=1)
                for w in range(ntile):
                    x_sb = x_all[:, w, :]
                    h_sb = h_all[:, w, :]
                    # packed GRU PSUM bank: rz[0:128] (accumulating), inp
                    # [128:192], hnp[192:256], xt[256:384], ht[384:512]
                    gp = psum.tile([128, 512], F32, tag="gp", name=f"gp_{step}_{w}")
                    nc.tensor.transpose(out=gp[:D, 256:384], in_=x_sb, identity=ident[:])
                    nc.tensor.transpose(out=gp[:D, 384:512], in_=h_sb, identity=ident[:])
                    xa = pool.tile([65, 128], BF16, tag="xa", name=f"xa_{step}_{w}", bufs=2)
                    nc.scalar.activation(xa[:D, :], gp[:D, 256:384],
                                         mybir.ActivationFunctionType.Copy)
                    nc.gpsimd.memset(xa[D:65, :], 1.0)
                    ha = pool.tile([65, 128], BF16, tag="ha", name=f"ha_{step}_{w}", bufs=2)
                    nc.vector.tensor_copy(ha[:D, :], gp[:D, 384:512])
                    nc.gpsimd.memset(ha[D:65, :], 1.0)
                    mm1 = nc.tensor.matmul(gp[:, 0:128], lhsT=xa[:, :], rhs=wiT_t[:, 0:2 * D],
                                           start=True, stop=False)
                    mm2 = nc.tensor.matmul(gp[:, 0:128], lhsT=ha[:, :], rhs=whT_t[:, 0:2 * D],
                                           start=False, stop=True)
                    add_dep_helper(mm2.ins, mm1.ins, reason="psum accum")
                    nc.tensor.matmul(gp[:, 128:192], lhsT=xa[:, :], rhs=wiT_t[:, 2 * D:],
                                     start=True, stop=True)
                    nc.tensor.matmul(gp[:, 192:256], lhsT=ha[:, :], rhs=whT_t[:, 2 * D:],
                                     start=True, stop=True)
                    rzs = pool.tile([128, 2 * D], F32, tag="rzs", name=f"rzs_{step}_{w}", bufs=2)
                    nc.scalar.activation(rzs[:, :], gp[:, 0:128],
                                         mybir.ActivationFunctionType.Sigmoid)
                    t1 = pool.tile([128, D], F32, tag="t1", name=f"t1_{step}_{w}", bufs=2)
                    nc.vector.tensor_mul(t1[:, :], rzs[:, :D], gp[:, 192:256])
                    nc.vector.tensor_add(t1[:, :], t1[:, :], gp[:, 128:192])
                    nn_ = pool.tile([128, D], F32, tag="nn", name=f"nn_{step}_{w}", bufs=2)
                    nc.scalar.activation(nn_[:, :], t1[:, :],
                                         mybir.ActivationFunctionType.Tanh)
                    # h' = n + z*(h - n)
                    t2 = pool.tile([128, D], F32, tag="t2", name=f"t2_{step}_{w}", bufs=2)
                    nc.vector.tensor_sub(t2[:, :], h_sb, nn_[:, :])
                    nc.vector.tensor_mul(t2[:, :], t2[:, :], rzs[:, D:])
                    nc.vector.tensor_add(hp_all[:, w, :], nn_[:, :], t2[:, :])
                if dst_h is not None:
                    bulk_store(hp_all, dst_h[:])
                if dst_out is not None:
                    bulk_store(hp_all, dst_out[:])

            # ---------------- main program
            gather_deps = [cc0]
            for s in range(STEPS):
                htab_ap = tables["x0"][:] if s == 0 else tables[(s - 1, "h")][:]
                hloc_ap = x0s[:] if s == 0 else h_shard[s - 1][:]
                message_substep(s, "bond", htab_ap, a_shard[(s, "a1")], gather_deps)
                cc1 = allgather(a_shard[(s, "a1")], tables[(s, "a1")])
                message_substep(s, "angle", tables[(s, "a1")][:],
                                a_shard[(s, "a2")], [cc1])
                cc2 = allgather(a_shard[(s, "a2")], tables[(s, "a2")])
                message_substep(s, "dihedral", tables[(s, "a2")][:],
                                a_shard[(s, "a3")], [cc2])
                gru_step(s, a_shard[(s, "a3")][:], hloc_ap,
                         h_shard[s] if s < STEPS - 1 else None,
                         out_shard if s == STEPS - 1 else None)
                if s < STEPS - 1:
                    cc3 = allgather(h_shard[s], tables[(s, "h")])
                    gather_deps = [cc3]

    lower_extended_insts(nc)
    _split_multi_waits(nc)
    return nc


# ------------------------------------------------------------------ public
def kernel(**inputs):
    af = np.asarray(inputs["atom_features"], dtype=np.float32)
    n_atoms = af.shape[0]
    shard = n_atoms // NCORES

    scheds, nchs, fdims, per_core = {}, {}, {}, {}
    spec = [
        ("bond", "bond_features", "pair_indices", "W_edge", "b_edge"),
        ("angle", "bond_angle_features", "bond_angle_pair_indices", "W_angle", "b_angle"),
        ("dihedral", "dihedral_angle_features", "dihedral_angle_pair_indices",
         "W_dihedral", "b_dihedral"),
    ]
    wrs = {}
    for t, fk, ik, wk, bk in spec:
        feat = np.asarray(inputs[fk], dtype=np.float32)
        pi = np.asarray(inputs[ik])
        fdims[t] = feat.shape[1]
        sched, pc, nch, util = _prep_type(pi, feat, n_atoms, shard, W_MAX[t])
        scheds[t], per_core[t], nchs[t] = sched, pc, nch
        wrs[t] = _pack_wr(np.asarray(inputs[wk], np.float32),
                          np.asarray(inputs[bk], np.float32), fdims[t])
        if os.environ.get("KVERBOSE"):
            print(f"[{t}] windows={nch} util={util:.2f}")

    wi = np.asarray(inputs["gru_wi"], np.float32)   # [3h, h]
    wh = np.asarray(inputs["gru_wh"], np.float32)
    bi = np.asarray(inputs["gru_bi"], np.float32)
    bh = np.asarray(inputs["gru_bh"], np.float32)
    wiT = np.concatenate([wi.T, bi[None, :]], 0).astype(np.float16)  # [65, 192]
    whT = np.concatenate([wh.T, bh[None, :]], 0).astype(np.float16)
    iota = np.tile(np.arange(128, dtype=np.float16)[None, :], (128, 1))

    nc = _build(n_atoms, scheds, nchs, fdims)

    in_maps = []
    for c in range(NCORES):
        m = dict(
            x0_shard=np.ascontiguousarray(af[c * shard:(c + 1) * shard]),
            iota_in=iota,
            wiT=wiT, whT=whT,
        )
        for t in ("bond", "angle", "dihedral"):
            pc = per_core[t][c]
            m[f"{t}_idx"] = pc["idx"]
            m[f"{t}_F"] = pc["F"]
            m[f"{t}_T"] = pc["T"]
            m[f"{t}_Wr"] = wrs[t]
        in_maps.append(m)

    if os.environ.get("KBUILD_ONLY"):
        _last_results["nc"] = nc
        _last_results["in_maps"] = in_maps
        return np.zeros((n_atoms, D), dtype=np.float32)
    if os.environ.get("KTIME"):
        results = _run_timed(nc, in_maps)
    else:
        res = run_bass_kernel_spmd(nc, in_maps, list(range(NCORES)))
        _last_results["exec_time_ns"] = res.exec_time_ns
        results = res.results

    out = np.zeros((n_atoms, D), dtype=np.float32)
    for c in range(NCORES):
        out[c * shard:(c + 1) * shard] = np.asarray(
            results[c]["out_shard"]).astype(np.float32)
    return out


def _run_timed(nc, in_maps, n_iters=3):
    """Replicates bass2jax.run_bass_via_pjrt but with device-resident inputs
    and repeated execution so the min wall time approximates HW exec time."""
    import time
    import jax
    from jax.sharding import Mesh, PartitionSpec
    from jax.experimental.shard_map import shard_map
    from concourse import bass2jax
    from concourse.bass2jax import _bass_exec_p, partition_id_tensor

    bass2jax.install_neuronx_cc_hook()
    n_cores = NCORES
    partition_name = nc.partition_id_tensor.name if nc.partition_id_tensor else None
    in_names, out_names, out_avals, zero_outs = [], [], [], []
    for alloc in nc.m.functions[0].allocations:
        if not isinstance(alloc, mybir.MemoryLocationSet):
            continue
        name = alloc.memorylocations[0].name
        if alloc.kind == "ExternalInput":
            if name != partition_name:
                in_names.append(name)
        elif alloc.kind == "ExternalOutput":
            out_names.append(name)
            shape = tuple(alloc.tensor_shape)
            dtype = mybir.dt.np(alloc.dtype)
            out_avals.append(jax.core.ShapedArray(shape, dtype))
            zero_outs.append(np.zeros(shape, dtype))
    n_params = len(in_names)
    all_in_names = list(in_names) + list(out_names)
    if partition_name is not None:
        all_in_names.append(partition_name)

    def _body(*args):
        operands = list(args)
        if partition_name is not None:
            operands.append(partition_id_tensor())
        outs = _bass_exec_p.bind(
            *operands,
            out_avals=tuple(out_avals),
            in_names=tuple(all_in_names),
            out_names=tuple(out_names),
            lowering_input_output_aliases=(),
            sim_require_finite=True,
            sim_require_nnan=True,
            nc=nc,
        )
        return tuple(outs)

    devices = jax.devices()[:n_cores]
    mesh = Mesh(np.asarray(devices), ("core",))
    spec = PartitionSpec("core")
    in_specs = (spec,) * (n_params + len(out_names))
    sharded = jax.jit(shard_map(_body, mesh=mesh, in_specs=in_specs,
                                out_specs=(spec,) * len(out_names),
                                check_rep=False), keep_unused=True)
    concat_in = [np.concatenate([np.asarray(in_maps[c][nm]) for c in range(n_cores)], 0)
                 for nm in in_names]
    concat_zeros = [np.zeros((n_cores * z.shape[0], *z.shape[1:]), z.dtype)
                    for z in zero_outs]
    sh = jax.sharding.NamedSharding(mesh, spec)
    dev_in = [jax.device_put(a, sh) for a in concat_in + concat_zeros]
    out = sharded(*dev_in)
    jax.block_until_ready(out)
    times = []
    for _ in range(n_iters):
        t0 = time.perf_counter()
        out = sharded(*dev_in)
        jax.block_until_ready(out)
        times.append(time.perf_counter() - t0)
    _last_results["exec_time_ns"] = int(min(times) * 1e9)
    _last_results["times"] = times
    return [
        {nm: np.asarray(out[i]).reshape(n_cores, *out_avals[i].shape)[c]
         for i, nm in enumerate(out_names)}
        for c in range(n_cores)
    ]
